# revision 56
# baseline (speedup 1.0000x reference)
"""Cosine-sim multi-head attention on 8 trn2 NeuronCores.

Sharding: core c -> (batch b = c//2, head-half hg = c%2). Each core computes
QKV projections for its 6 heads, full attention over S=2048, and a partial
out-projection [S, 768] in fp16. Host sums the two partials per batch + bo.

Per-core pipeline (fp16 operands, fp32 PSUM accum):
  QKV:     hst [768,2048] fp16 x W.T chunks -> q/k/vT [384, 2048]; hst
           loaded in 4 column-block DMAs so matmuls start after ~2.5us;
           next pair's QKV/norms staged across 4 hooks inside attention
  norms:   per-bank chains (q-side hides under k-proj): block-ones matmul
           -> rn; per-head scale*log2e folded into the fp32 rn drain;
           rsqrt on GPSIMD via pow(rn, -0.5); DRAM-bounce broadcast
  scores:  log2-domain logits s2[j,i] = cos * scale_h * log2(e),
           [128,2,512] psum tiles (2 heads), double-buffered
  exp:     one ACT activation Exp(scale=ln2) per jc -> 2^s2, fp16
  PV-T:    transposed PV, out ctxT[i, dh+1] 65-wide streams; denominator is
           the ones-column; 8 accumulators packed 7+1 across 2 PSUM banks
           (start=True only on the first group per bank: zeroing is
           bank-granular)
  norm:    batched strided reciprocal + per-accum tensor_scalar drain
  transp:  PE transposes (identity) back to ctx [dh, i], deferred into the
           NEXT pair's attention (ic==1 hook) to avoid PE head-blocking
  outproj: ctx x Wo.T -> o fp16, interleaved per-block after the last
           pair's transposes
"""
import numpy as np
import ml_dtypes

import concourse.bass as bass
import concourse.bacc as bacc
import concourse.tile as tile
from concourse import mybir

F16 = mybir.dt.float16
F32 = mybir.dt.float32
EXP = mybir.ActivationFunctionType.Exp
POW = mybir.AluOpType.pow

B, S, D = 4, 2048, 768
H, DH = 12, 64
HPC = 6            # heads per core
NPAIR = 3          # head pairs per core (m-tiles of 128)
NJC = S // 128     # 16 j-chunks
NIC = S // 512     # 4 i-blocks
MAX_LOG_SCALE = float(np.log(1.0 / 0.01))
LN2 = float(np.log(2.0))
LOG2E = float(np.log2(np.e))
POOL_JCS = (5, 10, 15)  # j-chunks whose exp runs on GPSIMD

# ctxT accumulator column offsets: 7 accums at 65-pitch in bank0, 1 in bank1
ACC_OFF = [65 * k for k in range(7)] + [512]

_NC_CACHE = {}


def build_nc():
    nc = bacc.Bacc(None, target_bir_lowering=False, debug=False)

    hst = nc.dram_tensor("hst", [D, S], F16, kind="ExternalInput")
    wqt = nc.dram_tensor("wqt", [D, 384], F16, kind="ExternalInput")
    wkt = nc.dram_tensor("wkt", [D, 384], F16, kind="ExternalInput")
    wvt = nc.dram_tensor("wvt", [D, 384], F16, kind="ExternalInput")
    wot = nc.dram_tensor("wot", [384, D], F16, kind="ExternalInput")
    bq3 = nc.dram_tensor("bq3", [128, 3], F32, kind="ExternalInput")
    bk3 = nc.dram_tensor("bk3", [128, 3], F32, kind="ExternalInput")
    bv3 = nc.dram_tensor("bv3", [128, 3], F32, kind="ExternalInput")
    cq3 = nc.dram_tensor("cq3", [128, 3], F32, kind="ExternalInput")
    i2d = nc.dram_tensor("i2d", [128, 2], F16, kind="ExternalInput")
    idn = nc.dram_tensor("idn", [128, 128], F16, kind="ExternalInput")
    o = nc.dram_tensor("o", [S, D], F16, kind="ExternalOutput")

    with tile.TileContext(nc) as tc:
        import contextlib
        with contextlib.ExitStack() as ctx:
            const = ctx.enter_context(tc.tile_pool(name="const", bufs=1))
            work = ctx.enter_context(tc.tile_pool(name="work", bufs=2, space="PSUM"))
            praw = ctx.enter_context(tc.tile_pool(name="praw", bufs=2))
            kraw_p = ctx.enter_context(tc.tile_pool(name="kraw", bufs=2))
            vtp = ctx.enter_context(tc.tile_pool(name="vtp", bufs=2))
            sqp = ctx.enter_context(tc.tile_pool(name="sqp", bufs=2))
            qsp = ctx.enter_context(tc.tile_pool(name="qsp", bufs=2))
            ksp = ctx.enter_context(tc.tile_pool(name="ksp", bufs=2))
            vap = ctx.enter_context(tc.tile_pool(name="vap", bufs=2))
            rnp = ctx.enter_context(tc.tile_pool(name="rnp", bufs=2))
            rrp = ctx.enter_context(tc.tile_pool(name="rrp", bufs=2))
            bcp = ctx.enter_context(tc.tile_pool(name="bcp", bufs=4))
            cnp = ctx.enter_context(tc.tile_pool(name="cnp", bufs=3))
            ctp = ctx.enter_context(tc.tile_pool(name="ctp", bufs=2))
            rdp = ctx.enter_context(tc.tile_pool(name="rdp", bufs=2))
            dram = ctx.enter_context(tc.tile_pool(name="dram", bufs=4, space="DRAM"))

            # ---- constants ----
            # hst/wq interleaved per-chunk on HWDGE (QKV consumes in kc order);
            # off-critical-path consts go through GPSIMD SWDGE.
            hst_sb = const.tile([128, 6, S], F16)
            w_sbs = [const.tile([128, 6, 384], F16, tag=nm, name=nm)
                     for nm in ("wq", "wk", "wv")]
            hst_r = hst[:, :].rearrange("(c p) s -> p c s", p=128)
            nc.sync.dma_start(out=hst_sb[:, :, 0:512], in_=hst_r[:, :, 0:512])
            for c in range(6):
                nc.scalar.dma_start(out=w_sbs[0][:, c, :],
                                    in_=wqt[c * 128:(c + 1) * 128, :])
            b_sbs = []
            for name, bt in (("bq", bq3), ("bk", bk3), ("bv", bv3)):
                b_sb = const.tile([128, 3], F32, tag=name)
                nc.scalar.dma_start(out=b_sb, in_=bt[:, :])
                b_sbs.append(b_sb)
            cq_sb = const.tile([128, 3], F32, tag="cq")
            nc.scalar.dma_start(out=cq_sb, in_=cq3[:, :])
            nc.sync.dma_start(out=hst_sb[:, :, 512:1024],
                              in_=hst_r[:, :, 512:1024])
            i2_sb = const.tile([128, 2], F16, tag="i2")
            nc.scalar.dma_start(out=i2_sb, in_=i2d[:, :])
            id_sb = const.tile([128, 128], F16, tag="idn")
            nc.scalar.dma_start(out=id_sb, in_=idn[:, :])
            nc.sync.dma_start(out=hst_sb[:, :, 1024:1536],
                              in_=hst_r[:, :, 1024:1536])
            nc.sync.dma_start(out=hst_sb[:, :, 1536:2048],
                              in_=hst_r[:, :, 1536:2048])
            for c in range(6):
                nc.scalar.dma_start(out=w_sbs[1][:, c, :],
                                    in_=wkt[c * 128:(c + 1) * 128, :])
            wot_sb = const.tile([128, 3, D], F16)

            def load_wv_wot():
                for c in range(6):
                    nc.scalar.dma_start(out=w_sbs[2][:, c, :],
                                        in_=wvt[c * 128:(c + 1) * 128, :])
                for c in range(3):
                    nc.sync.dma_start(out=wot_sb[:, c, :],
                                      in_=wot[c * 128:(c + 1) * 128, :])
            two16 = const.tile([128, 1024], F16, tag="two16")
            nc.vector.memset(two16, 2.0)
            nh32 = const.tile([128, 1024], F32, tag="nh32")
            nc.vector.memset(nh32, -0.5)

            ctxns = []

            def proj(p, ti):
                w_sb, b_sb = w_sbs[ti], b_sbs[ti]
                pool = (praw, kraw_p, vtp)[ti]
                dest = pool.tile([128, S], F16, tag=f"t{ti}", name=f"t{ti}")
                for ib in range(4):
                    ps = work.tile([128, 512], F32, tag="work", name="qkv_ps")
                    i0 = ib * 512
                    for kc in range(6):
                        nc.tensor.matmul(
                            ps,
                            w_sb[:, kc, p * 128:(p + 1) * 128],
                            hst_sb[:, kc, i0:i0 + 512],
                            start=(kc == 0), stop=(kc == 5))
                    nc.vector.tensor_scalar(
                        out=dest[:, i0:i0 + 512],
                        in0=ps,
                        scalar1=b_sb[:, p:p + 1],
                        scalar2=None,
                        op0=mybir.AluOpType.add)
                return dest

            def norms_bank(p, src, bank):
                """Per-bank norm chain: sq, rn matmuls, drain(+scale for q),
                rsqrt on GPSIMD, DRAM-bounce broadcast -> r_bc."""
                rn = work.tile([128, 512], F32, tag="work", name=f"rn{bank}")
                for ib in range(4):
                    sq = sqp.tile([128, 512], F16, tag="sq")
                    nc.vector.tensor_mul(sq, src[:, ib * 512:(ib + 1) * 512],
                                         src[:, ib * 512:(ib + 1) * 512])
                    nc.tensor.matmul(rn[32 * ib:32 * ib + 2, :],
                                     i2_sb, sq,
                                     start=True, stop=True,
                                     tile_position=(0, 32 * ib))
                rn_sb = rnp.tile([128, 512], F32, tag=f"rn{bank}",
                                 name=f"rnsb{bank}")
                if bank == 0:
                    nc.vector.tensor_scalar(
                        out=rn_sb, in0=rn,
                        scalar1=cq_sb[:, p:p + 1], scalar2=None,
                        op0=mybir.AluOpType.mult)
                else:
                    nc.vector.tensor_copy(rn_sb, rn)
                rr = rrp.tile([128, 512], F16, tag=f"rr{bank}",
                              name=f"rr{bank}")
                nc.gpsimd.tensor_tensor(out=rr, in0=rn_sb,
                                        in1=nh32[:, 0:512], op=POW)
                r_dr = dram.tile([2, 4, 512], F16, tag=f"rd{bank}")
                pst = rr.ap[0][0]
                for hh in range(2):
                    src_o = bass.AP(tensor=rr.tensor,
                                    offset=rr.offset + hh * pst,
                                    ap=[[32 * pst, 4], [1, 512]])
                    nc.sync.dma_start(out=r_dr[hh, :, :], in_=src_o)
                r_bc = bcp.tile([128, S], F16, tag=f"rb{bank}",
                                name=f"rb{bank}")
                for hh in range(2):
                    col = r_dr[hh, :, :]
                    src_ = bass.AP(tensor=col.tensor, offset=col.offset,
                                   ap=[[0, 64]] + col.ap)
                    nc.sync.dma_start(
                        out=r_bc[hh * 64:(hh + 1) * 64, :].rearrange(
                            "p (a b) -> p a b", a=4),
                        in_=src_)
                return r_bc

            def norm_muls(qraw, kraw, rbs):
                qs = qsp.tile([128, S], F16, tag="qs")
                nc.vector.tensor_mul(qs, qraw, rbs[0])
                ks = ksp.tile([128, S], F16, tag="ks")
                nc.vector.tensor_mul(ks, kraw, rbs[1])
                return qs, ks

            def transpose_pair(ctxTn, ctxn, post_t=None):
                for t in range(16):
                    tr_ps = work.tile([128, 128], F16, tag="work", name="tr")
                    nc.tensor.matmul(tr_ps, ctxTn[:, t, :], id_sb,
                                     start=True, stop=True, is_transpose=True)
                    nc.vector.tensor_copy(ctxn[:, t * 128:(t + 1) * 128], tr_ps)
                    if post_t is not None:
                        post_t(t, ctxn)

            def build_va(vT):
                # va[p, h, c, 0:64] = v rows for head h, j-chunk c; col 64 = 1
                # One full [128,128] PE transpose per j-chunk gives [j, dh] for
                # both heads at once (head h = cols h*64:(h+1)*64).
                va = vap.tile([128, 2, NJC, 80], F16, tag="va")
                nc.vector.memset(va[:, :, :, 64:65], 1.0)
                for c in range(NJC):
                    tr_ps = work.tile([128, 128], F16, tag="work", name="vtr")
                    nc.tensor.matmul(tr_ps, vT[:, c * 128:(c + 1) * 128],
                                     id_sb, start=True, stop=True,
                                     is_transpose=True)
                    dst = bass.AP(tensor=va.tensor,
                                  offset=va.offset + c * 80,
                                  ap=[va.ap[0], [1280, 2], [1, 64]])
                    src = bass.AP(tensor=tr_ps.tensor, offset=tr_ps.offset,
                                  ap=[tr_ps.ap[0], [64, 2], [1, 64]])
                    nc.vector.tensor_copy(dst, src)
                return va

            def attention(p, qs, ks, va, scores, epool, stp, cpool, hooks=(),
                          post_t=None):
                ctxTn = ctp.tile([128, 16, 128], F16, tag="ctxTn")
                ctxn = cnp.tile([128, S], F16, tag="ctxn")
                for ic in range(NIC):
                    for hic, fn in hooks:
                        if ic == hic:
                            fn()
                    i0 = ic * 512
                    # 8 ctxT accums [128, 65] packed 7+1 into 2 banks
                    ctx_ps = cpool.tile([128, 1024], F32, tag="ctx")

                    for jc in range(NJC):
                        s_ps = scores.tile([128, 2, 512], F32, tag="s")
                        nc.tensor.matmul(s_ps[:, 0, :],
                                         ks[0:64, jc * 128:(jc + 1) * 128],
                                         qs[0:64, i0:i0 + 512],
                                         start=True, stop=True,
                                         tile_position=(0, 0))
                        nc.tensor.matmul(s_ps[:, 1, :],
                                         ks[64:128, jc * 128:(jc + 1) * 128],
                                         qs[64:128, i0:i0 + 512],
                                         start=True, stop=True,
                                         tile_position=(64, 0))
                        e_sb = epool.tile([128, 2, 512], F16, tag="e")
                        nc.scalar.activation(
                            e_sb.rearrange("p a b -> p (a b)"),
                            s_ps.rearrange("p a b -> p (a b)"),
                            EXP, scale=LN2)
                        for k in range(4):
                            for h in range(2):
                                g = k * 2 + h
                                off = ACC_OFF[g]
                                nc.tensor.matmul(
                                    ctx_ps[:, off:off + 65],
                                    e_sb[:, h, k * 128:(k + 1) * 128],
                                    va[:, h, jc, 0:65],
                                    start=(jc == 0 and g in (0, 7)),
                                    stop=(jc == NJC - 1),
                                    skip_group_check=True)

                    # normalize: rden = 1/denom-col, drain accums to ctxTn
                    rden = rdp.tile([128, 8], F32, tag="rden")
                    dsrc = bass.AP(tensor=ctx_ps.tensor, offset=ctx_ps.offset + 64,
                                   ap=[ctx_ps.ap[0], [65, 7]])
                    nc.vector.reciprocal(rden[:, 0:7], dsrc)
                    nc.vector.reciprocal(rden[:, 7:8], ctx_ps[:, 576:577])
                    for k in range(4):
                        for h in range(2):
                            off = ACC_OFF[k * 2 + h]
                            nc.vector.tensor_scalar(
                                out=ctxTn[:, ic * 4 + k, h * 64:(h + 1) * 64],
                                in0=ctx_ps[:, off:off + 64],
                                scalar1=rden[:, k * 2 + h:k * 2 + h + 1],
                                scalar2=None,
                                op0=mybir.AluOpType.mult)
                # PE transpose back to [dh, i]; for the last pair (no
                # prefetch hooks) spread per-ic so out-proj overlaps attention
                return ctxTn, ctxn

            with tc.tile_pool(name="scores", bufs=2, space="PSUM") as scores, \
                 tc.tile_pool(name="epool", bufs=8) as epool, \
                 tc.tile_pool(name="stp", bufs=2) as stp, \
                 tc.tile_pool(name="cpool", bufs=1, space="PSUM") as cpool:
                pending = {}
                qraw0 = proj(0, 0)
                rqb0 = norms_bank(0, qraw0, 0)
                kraw0 = proj(0, 1)
                rkb0 = norms_bank(0, kraw0, 1)
                load_wv_wot()
                vT0 = proj(0, 2)
                qs0, ks0 = norm_muls(qraw0, kraw0, (rqb0, rkb0))
                pending[0] = (qs0, ks0, build_va(vT0))

                def make_hooks(pn):
                    part = {}

                    def h0():
                        part["q"] = proj(pn, 0)

                    def h1():
                        part["rqb"] = norms_bank(pn, part["q"], 0)
                        part["k"] = proj(pn, 1)

                    def h2():
                        part["rkb"] = norms_bank(pn, part["k"], 1)
                        part["v"] = proj(pn, 2)

                    def h3():
                        qs_, ks_ = norm_muls(part["q"], part["k"],
                                             (part["rqb"], part["rkb"]))
                        pending[pn] = (qs_, ks_, build_va(part["v"]))
                    return ((0, h0), (1, h1), (2, h2), (3, h3))

                with tc.tile_pool(name="osb", bufs=3) as osb:
                    def outproj_st(st, ctxn2):
                        o_sb = osb.tile([128, D], F16, tag="osb")
                        pairs = ctxns
                        for nn in range(2):
                            o_ps = work.tile([128, 512], F32, tag="work",
                                             name="o_ps")
                            for p in range(NPAIR):
                                nc.tensor.matmul(
                                    o_ps[:, 0:384],
                                    pairs[p][:, st * 128:(st + 1) * 128],
                                    wot_sb[:, p, nn * 384:(nn + 1) * 384],
                                    start=(p == 0), stop=(p == NPAIR - 1))
                            if st >= 12:
                                nc.scalar.activation(
                                    o_sb[:, nn * 384:(nn + 1) * 384],
                                    o_ps[:, 0:384],
                                    mybir.ActivationFunctionType.Copy)
                            else:
                                nc.vector.tensor_copy(
                                    o_sb[:, nn * 384:(nn + 1) * 384],
                                    o_ps[:, 0:384])
                        nc.sync.dma_start(out=o[st * 128:(st + 1) * 128, :],
                                          in_=o_sb)

                    prev_tn = None
                    for p in range(NPAIR):
                        qs, ks, va = pending.pop(p)
                        hooks = list(make_hooks(p + 1)) if p + 1 < NPAIR else []
                        if prev_tn is not None:
                            tn_, cn_ = prev_tn
                            hooks.append(
                                (1, lambda tn=tn_, cn=cn_:
                                    transpose_pair(tn, cn)))
                        tn, cn = attention(p, qs, ks, va, scores, epool,
                                           stp, cpool, hooks=hooks)
                        ctxns.append(cn)
                        prev_tn = (tn, cn)
                    transpose_pair(*prev_tn, post_t=outproj_st)

    nc.compile()
    return nc


def _prep_core_inputs(inputs, b, hg):
    f16 = np.float16
    hs = inputs["hidden_states"]
    rows = slice(hg * 384, (hg + 1) * 384)
    scale6 = np.exp(np.minimum(
        inputs["logit_scale"].reshape(H)[hg * HPC:(hg + 1) * HPC],
        MAX_LOG_SCALE)).astype(np.float64)

    def b3(bias):
        return np.ascontiguousarray(bias[rows].reshape(3, 128).T).astype(np.float32)

    # cq: per-partition scale c = 1/(scale_h*log2e)^2 applied to rn_q
    cq = np.ones((128, 3), np.float32)
    for p in range(3):
        for ib in range(4):
            for hh in range(2):
                cq[32 * ib + hh, p] = 1.0 / (scale6[p * 2 + hh] * LOG2E) ** 2
    i2 = np.zeros((128, 2), f16)
    i2[0:64, 0] = 1.0
    i2[64:128, 1] = 1.0
    return {
        "hst": np.ascontiguousarray(hs[b].T).astype(f16),
        "wqt": np.ascontiguousarray(inputs["Wq"][rows].T).astype(f16),
        "wkt": np.ascontiguousarray(inputs["Wk"][rows].T).astype(f16),
        "wvt": np.ascontiguousarray(inputs["Wv"][rows].T).astype(f16),
        "wot": np.ascontiguousarray(inputs["Wo"][:, rows].T).astype(f16),
        "bq3": b3(inputs["bq"]),
        "bk3": b3(inputs["bk"]),
        "bv3": b3(inputs["bv"]),
        "cq3": cq,
        "i2d": i2,
        "idn": np.eye(128, dtype=f16),
    }


def kernel(**inputs):
    from concourse.bass_utils import run_bass_kernel_spmd
    inputs = {k: np.asarray(v) for k, v in inputs.items()}
    if "nc" not in _NC_CACHE:
        _NC_CACHE["nc"] = build_nc()
    nc = _NC_CACHE["nc"]
    in_maps = [_prep_core_inputs(inputs, c // 2, c % 2) for c in range(8)]
    res = run_bass_kernel_spmd(nc, in_maps, core_ids=list(range(8)))
    out = np.empty((B, S, D), np.float32)
    bo = inputs["bo"].astype(np.float32)
    for b in range(B):
        out[b] = (res.results[2 * b]["o"].astype(np.float32)
                  + res.results[2 * b + 1]["o"].astype(np.float32) + bo)
    return out
